# revision 76
# baseline (speedup 1.0000x reference)
"""Causal multi-head attention (b=2, n=2048, d=768, 12 heads) on 8 TRN2 NeuronCores.

Sharding: batch x head-group. Core c handles batch c//4 and heads 3*(c%4) .. 3*(c%4)+2.
Each core gets xT = x[b].T plus W.T column slices for its 3 heads, computes the
unnormalized attention output (transposed) plus softmax denominators; the host
divides, transposes, and concatenates slabs into the full [2, 2048, 768].

Per-core algorithm (everything transposed so softmax reductions ride on matmuls):
  qT/kT/vT = (W.T slice).T @ xT            TensorE, per 512-col span
  v_nat[j, m] = transpose(vT) + ones column -> stationary [128, 65] per j-tile
  per head, per 512-col i-span:
    sT[j, i] = kT_h[:, jtile].T @ qT[:, span]   (psum, causally skipped/sliced)
    p = exp(sT) unshifted (max causal score ~66 fits fp32), bf16; diagonal
        128-blocks multiplied by a 0/1 bf16 triangular mask
    av[0:65, span] += v_nat[jtile].T @ p    (row 64 accumulates sum(p) = denom)
  av -> DRAM; host computes (av[0:64]/av[64]).T per head.

Perf facts measured on this hardware (see also the HAM/tile_position notes):
  - PSUM bank = 512 fp32; matmul outputs stay within one bank
  - keep K=128 and a single 128x128 PE mode everywhere: 64x128 row-tiled pairs
    DO run concurrently but their LDWEIGHTS cannot hide behind same-row-group
    in-flight MMs (~175ns exposed per wall) and the mode mixing throttles the
    HAM clock gate to 1.2 GHz -- measured net LOSS vs plain 128-mode
  - fp16 matmuls: 1.0 cyc/row, b2b 512-col spacing ~215ns @2.4GHz (zero
    exposed overhead); fp16 8x more mantissa than bf16 at the same speed, so
    x/W/q/k/v ride fp16 (bf16 q/k fails the 2e-2 gate; p=exp needs bf16
    RANGE, e^66 overflows fp16)
  - f32r <256-col matmuls run 4x slow at full clock (SBUF fetch); fp16/bf16
    do not -- that killed the old f32r diagonal tiles
  - fp32r/bf16 dtype-mode mixing triggered a 3.4us half-clock HAM epoch;
    all-16-bit removed it. HAM also halves the clock for one ~3.4us epoch
    after any epoch with low PE utilization -- keep the PE dense (warmup
    matmuls on zeros while the first DMAs land)
  - ACT exp = 0.84ns/col + ~250ns/instr; attn2 pairs 2 full j-tiles per exp
    (one [128,2,512] instr) to halve the fixed cost. ACT paces attn(3)
    (~25us exp vs ~21us PE) -- v-projections + transposes of span s are
    deferred INTO attn(s) (only its diagonal j-tiles need them) as PE filler
  - filler granularity matters: strictly 1 proj-op per insertion point
    scattered lone transposes between attention matmuls (+34ns on every
    neighboring matmul from PE mode switches); transposes ride in bursts
  - XBAR dma-transpose (dma_start transpose=True) is correct ONLY for
    offset-0 dests (any dest col-offset or strided 3D out AP silently
    shifts data); and 48 of them measured +19% total time -- PE transposes win
  - DVE TensorTensor cannot touch PSUM (BIR verifier); Pool/GpSimd cannot
    read PSUM either (copies stay on DVE); masks ride bf16 SBUF
  - device clock varies ~10% between process-level "windows" (DVFS /
    shared infra); A/B within one process, min-of-N across runs
"""
import sys

if "/opt/trn_rl_repo" not in sys.path:
    sys.path.insert(0, "/opt/trn_rl_repo")

from contextlib import ExitStack

import ml_dtypes
import numpy as np

import concourse.bass as bass
import concourse.tile as tile
from concourse import bacc, mybir, bass_utils
from concourse.masks import make_identity

F32 = mybir.dt.float32
F32R = mybir.dt.float32r
BF16 = mybir.dt.bfloat16
FP16 = mybir.dt.float16

P = 128
H = 64
SPAN = 512
HD = 64

B, N, D, NH = 2, 2048, 768, 12
HL = 3                       # heads per core
DL = HL * HD                 # 192
N_CORES = 8
KT = D // P                  # 6 contraction chunks
KH = KT // 2                 # kt per x/w half
NS = N // SPAN               # 4 spans
NT = N // P                  # 16 j-tiles
CPS = SPAN // P              # 4 chunks per span

DT_PROJ = FP16               # x, W, qT/kT/vT
DT_P = BF16                  # p = exp(scores), v_nat
WARMUP_N = 5                 # before first projection (cover to first x/w DMA)
WARMUP_MID = 4               # between kt halves of chunk 0 (cover to x00b)
EXP = mybir.ActivationFunctionType.Exp


def _build(nc, tc, dt_proj, dt_p):
    # host pre-packs: xt[p, (ns, kth, ktl, c)], wc[p, (kt, m)] with
    # m = packed weight columns [q01 | k01 | v01 | k2+v2 | q2]
    xt = nc.dram_tensor("xt", [P, N * KT], dt_proj, kind="ExternalInput").ap()
    wc = nc.dram_tensor("wc", [P, KT * 3 * DL], dt_proj,
                        kind="ExternalInput").ap()
    o = nc.dram_tensor("o", [HL * (HD + 1), N], F32, kind="ExternalOutput").ap()

    with ExitStack() as ctx:
        pool = lambda name, bufs, **kw: ctx.enter_context(
            tc.tile_pool(name=name, bufs=bufs, **kw))
        const_pool = pool("const", 1)
        xpool = pool("x", 2 * NS)
        wpool = pool("w", 2)
        qk_pool = pool("qk", NS)
        kz_pool = pool("kz", HL * NS)
        vnat_pool = pool("vnat", 1)
        ppool = pool("p", 6)
        osb_pool = pool("osb", 3)
        ps = pool("ps", 2, space="PSUM")        # [128,1024] sc pair tiles: 2x2 banks
        ps_pj = pool("ps_pj", 2, space="PSUM")  # [128,512] proj/transpose: 2x1 bank
        ps_av = pool("ps_av", 2, space="PSUM")  # [65,512] accumulators: 2x1 bank

        # zeros_r first: it is the only dependency of the warmup matmuls, so
        # the PE can start ~5us in (right after the engine-start barrier)
        zeros_r = const_pool.tile([P, SPAN], dt_proj)
        nc.gpsimd.memset(zeros_r[:], 0.0)
        ident = const_pool.tile([P, P], F32)
        make_identity(nc, ident[:])
        ident_r = const_pool.tile([P, P], dt_proj)
        nc.vector.tensor_copy(ident_r[:], ident[:])
        # multiplicative causal mask for [key-partition, query-col] diag blocks:
        # 1 where key <= query, 0 where key > query (bf16, post-exp multiply)
        tri16 = const_pool.tile([P, P], dt_p)
        nc.gpsimd.memset(tri16[:], 0.0)
        nc.gpsimd.affine_select(
            out=tri16[:], in_=tri16[:], compare_op=mybir.AluOpType.is_gt,
            fill=1.0, base=0, pattern=[[-1, P]], channel_multiplier=1)

        # ---- DMA inputs: w halves + 8 x chunks (span, kt-half) ----
        x_tiles = [xpool.tile([P, KH * SPAN], dt_proj, tag="x", name=f"x{i}")
                   for i in range(2 * NS)]

        def x_slice(ns, kt):
            t = x_tiles[2 * ns + kt // KH]
            b = (kt % KH) * SPAN
            return t[:, b:b + SPAN]

        def dma_x(ns, half):
            w = KH * SPAN
            i = 2 * ns + half
            nc.sync.dma_start(x_tiles[i][:], xt[:, i * w:(i + 1) * w])

        # weights packed per m-chunk: wc columns = (chunk, kt, m) so chunk 0
        # only gates on its own 3KB slice (ready ~11us) and the x halves
        CH_W = (P, P, P, P, HD)
        CH_OFF = [sum(CH_W[:i]) for i in range(len(CH_W) + 1)]
        w_ch = [wpool.tile([P, KT * CH_W[ci]], dt_proj, tag=f"w{ci}",
                           name=f"w{ci}") for ci in range(5)]

        def w_slice(ci, kt):
            msz = CH_W[ci]
            return w_ch[ci][:, kt * msz:(kt + 1) * msz]

        def dma_w(ci):
            b = KT * CH_OFF[ci]
            w = KT * CH_W[ci]
            nc.sync.dma_start(w_ch[ci][:], wc[:, b:b + w])

        dma_w(0)
        dma_x(0, 0)
        dma_x(0, 1)
        for ci in range(1, 5):
            dma_w(ci)
        for ns in range(1, NS):
            dma_x(ns, 0)
            dma_x(ns, 1)

        # ---- per-span q/k/v tiles (zero-padded K=128 layout) ----
        qT01 = [qk_pool.tile([P, SPAN], dt_proj, tag="q01", name=f"q01_{i}") for i in range(NS)]
        qT2z = [qk_pool.tile([P, SPAN], dt_proj, tag="q2z", name=f"q2z_{i}") for i in range(NS)]
        vT01 = [qk_pool.tile([P, SPAN], dt_proj, tag="v01", name=f"v01_{i}") for i in range(NS)]
        vT2z = [qk_pool.tile([P, SPAN], dt_proj, tag="v2z", name=f"v2z_{i}") for i in range(NS)]
        kTz = [[kz_pool.tile([P, SPAN], dt_proj, tag="kz", name=f"kz_{h}_{i}")
                for i in range(NS)] for h in range(HL)]

        def zfill(ap):
            nc.gpsimd.memset(ap, 0.0)

        for ns in range(NS):
            zfill(qT2z[ns][HD:P, :])
            zfill(vT2z[ns][0:HD, :])
            zfill(kTz[0][ns][HD:P, :])
            zfill(kTz[1][ns][0:HD, :])
            zfill(kTz[2][ns][HD:P, :])

        # v natural layout: heads 0,1 interleaved per j-tile [v0|1|v1|1], head 2
        # separate [v2|1]; the ones column accumulates the softmax denominator.
        v_nat01 = vnat_pool.tile([P, NT * 2 * (HD + 1)], dt_p, tag="vnat01")
        v_nat2 = vnat_pool.tile([P, NT * (HD + 1)], dt_p, tag="vnat2")
        c01 = v_nat01[:].rearrange("p (t c) -> p t c", c=HD + 1)[:, :, HD]
        c2 = v_nat2[:].rearrange("p (t c) -> p t c", c=HD + 1)[:, :, HD]
        nc.gpsimd.memset(c01, 1.0)
        nc.gpsimd.memset(c2, 1.0)

        def vnat(h, jt):
            if h < 2:
                b = jt * 2 * (HD + 1) + h * (HD + 1)
                return v_nat01[:, b:b + HD + 1]
            b = jt * (HD + 1)
            return v_nat2[:, b:b + HD + 1]

        # ---- warmup: keep the PE busy while the first DMAs land ----
        warm = ps_pj.tile([P, SPAN], F32, tag="ps_pj", name="warm")

        def warmup(n):
            for _ in range(n):
                nc.tensor.matmul(warm[:], zeros_r[:, 0:P], zeros_r[:],
                                 start=True, stop=True)

        warmup(WARMUP_N)

        # ---- projections: qk chunks (needed before attn(s) starts) and v
        # chunks + transposes (needed only by attn(s)'s DIAGONAL j-tiles,
        # so they can fill deep into attn(s) itself) ----
        m_chunks = ((0, P, "q01"), (1, P, "k01"), (2, P, "k2q2"),
                    (3, P, "v01"), (4, HD, "v2"))

        def chunk_op(ns, ci, msz, what, midfill=0):
            pt = ps_pj.tile([msz, SPAN], F32, tag="ps_pj", name=f"pj_{ns}_{what}")
            for kt in range(KT):
                if midfill and kt == KH:
                    warmup(midfill)
                nc.tensor.matmul(
                    pt[:], w_slice(ci, kt), x_slice(ns, kt),
                    start=(kt == 0), stop=(kt == KT - 1))
            if what == "q01":
                nc.vector.tensor_copy(qT01[ns][:], pt[:])
            elif what == "k01":
                nc.vector.tensor_copy(kTz[0][ns][0:HD, :], pt[0:HD, :])
                nc.vector.tensor_copy(kTz[1][ns][HD:P, :], pt[HD:P, :])
            elif what == "k2q2":
                nc.vector.tensor_copy(kTz[2][ns][0:HD, :], pt[0:HD, :])
                nc.vector.tensor_copy(qT2z[ns][0:HD, :], pt[HD:P, :])
            elif what == "v01":
                nc.vector.tensor_copy(vT01[ns][:], pt[:])
            else:
                nc.vector.tensor_copy(vT2z[ns][HD:P, :], pt[:])

        def transp01_op(ns, c):
            jt = ns * CPS + c
            tp = ps_pj.tile([P, P], dt_proj, tag="ps_pj", name=f"tp_{jt}")
            nc.tensor.transpose(tp[:], vT01[ns][:, c * P:(c + 1) * P],
                                ident_r[:])
            nc.vector.tensor_copy(
                v_nat01[:].rearrange("p (t c) -> p t c", c=HD + 1)[
                    :, 2 * jt:2 * jt + 2, 0:HD],
                tp[:].rearrange("p (t c) -> p t c", c=HD))

        def transp2_op(ns, c):
            jt = ns * CPS + c
            tp2 = ps_pj.tile([P, P], dt_proj, tag="ps_pj", name=f"tp2_{jt}")
            nc.tensor.transpose(tp2[:], vT2z[ns][:, c * P:(c + 1) * P],
                                ident_r[:])
            nc.vector.tensor_copy(
                v_nat2[:, jt * (HD + 1):jt * (HD + 1) + HD], tp2[:, HD:P])

        def qk_groups(ns):
            return [[lambda a=ci, b=msz, w=what: chunk_op(ns, a, b, w)]
                    for (ci, msz, what) in m_chunks[:3]]

        def v_groups(ns):
            # transposes ride in bursts of 4: a lone transpose between
            # attention matmuls costs two PE mode switches (measured +34ns
            # on every neighboring matmul when fully scattered)
            gs = [[lambda a=ci, b=msz, w=what: chunk_op(ns, a, b, w)]
                  for (ci, msz, what) in m_chunks[3:]]
            for c0 in (0, 2):
                gs.append([lambda c=c, t=t: t(ns, c) for c in (c0, c0 + 1)
                           for t in (transp01_op, transp2_op)])
            return gs

        # due-point scheduler: each deferred proj GROUP gets an absolute pav
        # insertion-point index; drained at-or-before that point. Points:
        # attn(0): 0-7, attn(1): 8-21, attn(2): 22-41, attn(3): 42-67.
        sched = []
        point = [0]

        def add_filler(groups, p0, p1):
            n = len(groups)
            for j, g in enumerate(groups):
                sched.append((p0 + (p1 - p0) * j // n, g))

        def drain_even():
            while sched and sched[0][0] <= point[0]:
                for op in sched.pop(0)[1]:
                    op()
            point[0] += 1

        def finalize(s, h, av):
            ob = osb_pool.tile([HD + 1, SPAN], F32, tag="osb", name=f"ob{s}_{h}")
            nc.vector.tensor_copy(ob[:], av[:])
            nc.sync.dma_start(
                o[h * (HD + 1):(h + 1) * (HD + 1),
                  s * SPAN:(s + 1) * SPAN], ob[:])

        # ---- attention: heads 0,1 fused pair loop; head 2 solo ----
        def jt_order(s, njt):
            # sequential: full tiles stream b2b, diag chains at the end
            # (interleaving diags mid-stream measured WORSE: +6.7us of
            # pipeline gaps in spans 2-3)
            return list(range(njt))

        def attn01(s):
            njt = CPS * (s + 1)
            order = jt_order(s, njt)
            av0 = ps_av.tile([HD + 1, SPAN], F32, tag="ps_av", name=f"av0_{s}")
            av1 = ps_av.tile([HD + 1, SPAN], F32, tag="ps_av", name=f"av1_{s}")
            live = {}

            def emit_sc(jt):
                c_d = jt - CPS * s
                n0 = max(c_d, 0) * P
                ns_k, ck = jt // CPS, jt % CPS
                sc = ps.tile([P, 2 * SPAN], F32, tag="ps", name=f"sc01_{s}_{jt}")
                nc.tensor.matmul(sc[:, n0:SPAN],
                                 kTz[0][ns_k][:, ck * P:(ck + 1) * P],
                                 qT01[s][:, n0:SPAN], start=True, stop=True)
                nc.tensor.matmul(sc[:, SPAN + n0:2 * SPAN],
                                 kTz[1][ns_k][:, ck * P:(ck + 1) * P],
                                 qT01[s][:, n0:SPAN], start=True, stop=True)
                live[jt] = (sc, n0, c_d >= 0)

            def emit_pav(i):
                jt = order[i]
                sc, n0, diag = live.pop(jt)
                p = ppool.tile([P, 2 * SPAN], dt_p, tag="p", name=f"p01_{s}_{jt}")
                sc3 = sc[:].rearrange("q (t c) -> q t c", c=SPAN)
                p3 = p[:].rearrange("q (t c) -> q t c", c=SPAN)
                nc.scalar.activation(p3[:, :, n0:SPAN], sc3[:, :, n0:SPAN], EXP)
                if diag:
                    nc.vector.tensor_mul(p[:, n0:n0 + P], p[:, n0:n0 + P],
                                         tri16[:])
                    nc.vector.tensor_mul(
                        p[:, SPAN + n0:SPAN + n0 + P],
                        p[:, SPAN + n0:SPAN + n0 + P], tri16[:])
                # safe insertion point: every live sc tile's reader is emitted;
                # proj bursts and the two-ahead sc keep Tensor fed through the
                # exp latency
                drain_even()
                if i + 2 < njt and order[i + 2] not in live:
                    emit_sc(order[i + 2])
                st, sp = (i == 0), (i == njt - 1)
                nc.tensor.matmul(av0[:, n0:SPAN], vnat(0, jt), p[:, n0:SPAN],
                                 start=st, stop=sp)
                nc.tensor.matmul(av1[:, n0:SPAN], vnat(1, jt),
                                 p[:, SPAN + n0:2 * SPAN], start=st, stop=sp)

            emit_sc(order[0])
            if njt > 1:
                emit_sc(order[1])
            for i in range(njt):
                emit_pav(i)
            finalize(s, 0, av0)
            finalize(s, 1, av1)

        def attn2(s):
            # head-2 stream: full tiles processed in PAIRS sharing one
            # [128, 2*SPAN] psum tile and ONE exp instruction (halves the
            # ACT fixed cost); diagonal tiles stay single, woven between
            njt = CPS * (s + 1)
            full = list(range(CPS * s))
            diag = list(range(CPS * s, njt))
            units = [(full[i], full[i + 1]) for i in range(0, len(full), 2)]
            # diag j-tiles pair up too: one exp over the union column range
            # [n0_first:512] per half; the second tile's [n0_first:n0_own)
            # slice exps stale psum (finite old scores) that no av reads
            units.extend(((diag[0], diag[1]), (diag[2], diag[3])))
            av2 = ps_av.tile([HD + 1, SPAN], F32, tag="ps_av", name=f"av2_{s}")
            live = {}

            def emit_sc(u):
                sc = ps.tile([P, 2 * SPAN], F32, tag="ps",
                             name=f"sc2_{s}_{u[0]}")
                n0f = SPAN
                for t, jt in enumerate(u):
                    c_d = jt - CPS * s
                    n0 = max(c_d, 0) * P
                    n0f = min(n0f, n0)
                    ns_k, ck = jt // CPS, jt % CPS
                    nc.tensor.matmul(sc[:, t * SPAN + n0:(t + 1) * SPAN],
                                     kTz[2][ns_k][:, ck * P:(ck + 1) * P],
                                     qT2z[s][:, n0:SPAN], start=True, stop=True)
                live[u] = (sc, n0f)

            def emit_pav(i):
                u = units[i]
                sc, n0, = live.pop(u)
                p = ppool.tile([P, 2 * SPAN], dt_p, tag="p",
                               name=f"p2_{s}_{u[0]}")
                sc3 = sc[:].rearrange("q (t c) -> q t c", c=SPAN)
                p3 = p[:].rearrange("q (t c) -> q t c", c=SPAN)
                nc.scalar.activation(p3[:, :, n0:SPAN], sc3[:, :, n0:SPAN],
                                     EXP)
                for t, jt in enumerate(u):
                    c_d = jt - CPS * s
                    if c_d >= 0:
                        tn0 = t * SPAN + c_d * P
                        nc.vector.tensor_mul(p[:, tn0:tn0 + P],
                                             p[:, tn0:tn0 + P], tri16[:])
                drain_even()
                if i + 2 < len(units) and units[i + 2] not in live:
                    emit_sc(units[i + 2])
                for t, jt in enumerate(u):
                    c_d = jt - CPS * s
                    tn0 = max(c_d, 0) * P
                    st = (i == 0 and t == 0)
                    sp = (i == len(units) - 1 and t == len(u) - 1)
                    nc.tensor.matmul(av2[:, tn0:SPAN], vnat(2, jt),
                                     p[:, t * SPAN + tn0:(t + 1) * SPAN],
                                     start=st, stop=sp)

            emit_sc(units[0])
            if len(units) > 1:
                emit_sc(units[1])
            for i in range(len(units)):
                emit_pav(i)
            finalize(s, 2, av2)

        # span 0 projects standalone (warmup mid-fill covers the kt0-2 ->
        # kt3-5 x-DMA boundary of the first chunk); everything later drains
        # into the attention phases as PE filler, as late as its consumers
        # allow -- qk(s) before attn(s) starts, v(s)+transposes(s) before
        # attn(s)'s first DIAGONAL av -- so the ACT-paced late spans stay
        # PE-dense
        first = True
        for (ci, msz, what) in m_chunks:
            chunk_op(0, ci, msz, what, midfill=WARMUP_MID if first else 0)
            first = False
        for c in range(CPS):
            transp01_op(0, c)
            transp2_op(0, c)
        add_filler(qk_groups(1), 0, 3)
        add_filler(v_groups(1), 3, 12)
        add_filler(qk_groups(2), 12, 19)
        add_filler(v_groups(2), 19, 30)
        add_filler(qk_groups(3), 30, 39)
        add_filler(v_groups(3), 39, 53)
        for s in range(NS):
            attn01(s)
            attn2(s)
        while sched:
            for op in sched.pop(0)[1]:
                op()


_NC_CACHE = {}


def _get_module(dt_proj=DT_PROJ, dt_p=DT_P):
    key = (dt_proj, dt_p)
    if key not in _NC_CACHE:
        nc = bacc.Bacc("TRN2", target_bir_lowering=False, debug=False)
        with tile.TileContext(nc) as tc:
            _build(nc, tc, dt_proj, dt_p)
        nc.compile()
        _NC_CACHE[key] = nc
    return _NC_CACHE[key]


def _in_maps(x, Wq, Wk, Wv):
    maps = []
    xT = [np.ascontiguousarray(
        x[b].T.reshape(KT, P, NS, SPAN).transpose(1, 2, 0, 3).reshape(P, -1)
        .astype(np.float16))
        for b in range(B)]
    WqT, WkT, WvT = Wq.T, Wk.T, Wv.T
    for c in range(N_CORES):
        bc, g = divmod(c, N_CORES // B)
        s0 = g * DL
        wcomb = np.concatenate([
            WqT[:, s0:s0 + P], WkT[:, s0:s0 + P],
            WkT[:, s0 + P:s0 + DL], WqT[:, s0 + P:s0 + DL],
            WvT[:, s0:s0 + P], WvT[:, s0 + P:s0 + DL]], axis=1)
        # pack as (chunk, kt, m): per m-chunk, kt-major
        w3 = wcomb.reshape(KT, P, 3 * DL).transpose(1, 0, 2)  # [P, kt, m]
        parts = []
        for c0, c1 in ((0, P), (P, 2 * P), (2 * P, 3 * P), (3 * P, 4 * P),
                       (4 * P, 4 * P + HD)):
            parts.append(w3[:, :, c0:c1].reshape(P, -1))
        wpk = np.ascontiguousarray(
            np.concatenate(parts, axis=1).astype(np.float16))
        maps.append({
            "xt": xT[bc],
            "wc": wpk,
        })
    return maps


def kernel(x, Wq, Wk, Wv, _trace=False, _tmpdir=None, **_kw):
    x = np.asarray(x, dtype=np.float32)
    Wq = np.asarray(Wq, dtype=np.float32)
    Wk = np.asarray(Wk, dtype=np.float32)
    Wv = np.asarray(Wv, dtype=np.float32)
    assert x.shape == (B, N, D) and Wq.shape == (D, D)

    nc = _get_module()
    res = bass_utils.run_bass_kernel_spmd(
        nc, _in_maps(x, Wq, Wk, Wv), core_ids=list(range(N_CORES)),
        trace=_trace, tmpdir=_tmpdir)
    out = np.empty((B, N, D), np.float32)
    for c in range(N_CORES):
        bc, g = divmod(c, N_CORES // B)
        oT = res.results[c]["o"].astype(np.float64)
        for h in range(HL):
            blk = oT[h * (HD + 1):h * (HD + 1) + HD, :]
            den = oT[h * (HD + 1) + HD, :]
            out[bc, :, g * DL + h * HD:g * DL + (h + 1) * HD] = \
                (blk / den).T.astype(np.float32)
    if _trace:
        return out, res
    return out



# revision 78
# speedup vs baseline: 1.0096x; 1.0096x over previous
"""Causal multi-head attention (b=2, n=2048, d=768, 12 heads) on 8 TRN2 NeuronCores.

Sharding: batch x head-group. Core c handles batch c//4 and heads 3*(c%4) .. 3*(c%4)+2.
Each core gets xT = x[b].T plus W.T column slices for its 3 heads, computes the
unnormalized attention output (transposed) plus softmax denominators; the host
divides, transposes, and concatenates slabs into the full [2, 2048, 768].

Per-core algorithm (everything transposed so softmax reductions ride on matmuls):
  qT/kT/vT = (W.T slice).T @ xT            TensorE, per 512-col span
  v_nat[j, m] = transpose(vT) + ones column -> stationary [128, 65] per j-tile
  per head, per 512-col i-span:
    sT[j, i] = kT_h[:, jtile].T @ qT[:, span]   (psum, causally skipped/sliced)
    p = exp(sT) unshifted (max causal score ~66 fits fp32), bf16; diagonal
        128-blocks multiplied by a 0/1 bf16 triangular mask
    av[0:65, span] += v_nat[jtile].T @ p    (row 64 accumulates sum(p) = denom)
  av -> DRAM; host computes (av[0:64]/av[64]).T per head.

Perf facts measured on this hardware (see also the HAM/tile_position notes):
  - PSUM bank = 512 fp32; matmul outputs stay within one bank
  - keep K=128 and a single 128x128 PE mode everywhere: 64x128 row-tiled pairs
    DO run concurrently but their LDWEIGHTS cannot hide behind same-row-group
    in-flight MMs (~175ns exposed per wall) and the mode mixing throttles the
    HAM clock gate to 1.2 GHz -- measured net LOSS vs plain 128-mode
  - fp16 matmuls: 1.0 cyc/row, b2b 512-col spacing ~215ns @2.4GHz (zero
    exposed overhead); fp16 8x more mantissa than bf16 at the same speed, so
    x/W/q/k/v ride fp16 (bf16 q/k fails the 2e-2 gate; p=exp needs bf16
    RANGE, e^66 overflows fp16)
  - f32r <256-col matmuls run 4x slow at full clock (SBUF fetch); fp16/bf16
    do not -- that killed the old f32r diagonal tiles
  - fp32r/bf16 dtype-mode mixing triggered a 3.4us half-clock HAM epoch;
    all-16-bit removed it. HAM also halves the clock for one ~3.4us epoch
    after any epoch with low PE utilization -- keep the PE dense (warmup
    matmuls on zeros while the first DMAs land)
  - ACT exp = 0.84ns/col + ~250ns/instr; attn2 pairs 2 full j-tiles per exp
    (one [128,2,512] instr) to halve the fixed cost. ACT paces attn(3)
    (~25us exp vs ~21us PE) -- v-projections + transposes of span s are
    deferred INTO attn(s) (only its diagonal j-tiles need them) as PE filler
  - filler granularity matters: strictly 1 proj-op per insertion point
    scattered lone transposes between attention matmuls (+34ns on every
    neighboring matmul from PE mode switches); transposes ride in bursts
  - XBAR dma-transpose (dma_start transpose=True) is correct ONLY for
    offset-0 dests (any dest col-offset or strided 3D out AP silently
    shifts data); and 48 of them measured +19% total time -- PE transposes win
  - DVE TensorTensor cannot touch PSUM (BIR verifier); Pool/GpSimd cannot
    read PSUM either (copies stay on DVE); masks ride bf16 SBUF
  - device clock varies ~10% between process-level "windows" (DVFS /
    shared infra); A/B within one process, min-of-N across runs
"""
import sys

if "/opt/trn_rl_repo" not in sys.path:
    sys.path.insert(0, "/opt/trn_rl_repo")

from contextlib import ExitStack

import ml_dtypes
import numpy as np

import concourse.bass as bass
import concourse.tile as tile
from concourse import bacc, mybir, bass_utils
from concourse.masks import make_identity

F32 = mybir.dt.float32
F32R = mybir.dt.float32r
BF16 = mybir.dt.bfloat16
FP16 = mybir.dt.float16

P = 128
H = 64
SPAN = 512
HD = 64

B, N, D, NH = 2, 2048, 768, 12
HL = 3                       # heads per core
DL = HL * HD                 # 192
N_CORES = 8
KT = D // P                  # 6 contraction chunks
KH = KT // 2                 # kt per x/w half
NS = N // SPAN               # 4 spans
NT = N // P                  # 16 j-tiles
CPS = SPAN // P              # 4 chunks per span

DT_PROJ = FP16               # x, W, qT/kT/vT
DT_P = BF16                  # p = exp(scores), v_nat
WARMUP_N = 5                 # before first projection (cover to first x/w DMA)
WARMUP_MID = 4               # between kt halves of chunk 0 (cover to x00b)
EXP = mybir.ActivationFunctionType.Exp


def _build(nc, tc, dt_proj, dt_p):
    # host pre-packs: xt[p, (ns, kth, ktl, c)], wc[p, (kt, m)] with
    # m = packed weight columns [q01 | k01 | v01 | k2+v2 | q2]
    xt = nc.dram_tensor("xt", [P, N * KT], dt_proj, kind="ExternalInput").ap()
    wc = nc.dram_tensor("wc", [P, KT * 3 * DL], dt_proj,
                        kind="ExternalInput").ap()
    o = nc.dram_tensor("o", [HL * (HD + 1), N], F32, kind="ExternalOutput").ap()

    with ExitStack() as ctx:
        pool = lambda name, bufs, **kw: ctx.enter_context(
            tc.tile_pool(name=name, bufs=bufs, **kw))
        const_pool = pool("const", 1)
        xpool = pool("x", 2 * NS)
        wpool = pool("w", 2)
        qk_pool = pool("qk", NS)
        kz_pool = pool("kz", HL * NS)
        vnat_pool = pool("vnat", 1)
        ppool = pool("p", 6)
        osb_pool = pool("osb", 3)
        ps = pool("ps", 2, space="PSUM")        # [128,1024] sc pair tiles: 2x2 banks
        ps_pj = pool("ps_pj", 2, space="PSUM")  # [128,512] proj/transpose: 2x1 bank
        ps_av = pool("ps_av", 2, space="PSUM")  # [65,512] accumulators: 2x1 bank

        # zeros_r first: it is the only dependency of the warmup matmuls, so
        # the PE can start ~5us in (right after the engine-start barrier)
        zeros_r = const_pool.tile([P, SPAN], dt_proj)
        nc.gpsimd.memset(zeros_r[:], 0.0)
        ident = const_pool.tile([P, P], F32)
        make_identity(nc, ident[:])
        ident_r = const_pool.tile([P, P], dt_proj)
        nc.vector.tensor_copy(ident_r[:], ident[:])
        # multiplicative causal mask for [key-partition, query-col] diag blocks:
        # 1 where key <= query, 0 where key > query (bf16, post-exp multiply)
        tri16 = const_pool.tile([P, P], dt_p)
        nc.gpsimd.memset(tri16[:], 0.0)
        nc.gpsimd.affine_select(
            out=tri16[:], in_=tri16[:], compare_op=mybir.AluOpType.is_gt,
            fill=1.0, base=0, pattern=[[-1, P]], channel_multiplier=1)

        # ---- DMA inputs: w halves + 8 x chunks (span, kt-half) ----
        x_tiles = [xpool.tile([P, KH * SPAN], dt_proj, tag="x", name=f"x{i}")
                   for i in range(2 * NS)]

        def x_slice(ns, kt):
            t = x_tiles[2 * ns + kt // KH]
            b = (kt % KH) * SPAN
            return t[:, b:b + SPAN]

        def dma_x(ns, half):
            w = KH * SPAN
            i = 2 * ns + half
            nc.sync.dma_start(x_tiles[i][:], xt[:, i * w:(i + 1) * w])

        # weights packed per m-chunk: wc columns = (chunk, kt, m) so chunk 0
        # only gates on its own 3KB slice (ready ~11us) and the x halves
        CH_W = (P, P, P, P, HD)
        CH_OFF = [sum(CH_W[:i]) for i in range(len(CH_W) + 1)]
        w_ch = [wpool.tile([P, KT * CH_W[ci]], dt_proj, tag=f"w{ci}",
                           name=f"w{ci}") for ci in range(5)]

        def w_slice(ci, kt):
            msz = CH_W[ci]
            return w_ch[ci][:, kt * msz:(kt + 1) * msz]

        def dma_w(ci):
            b = KT * CH_OFF[ci]
            w = KT * CH_W[ci]
            nc.sync.dma_start(w_ch[ci][:], wc[:, b:b + w])

        dma_w(0)
        dma_x(0, 0)
        dma_x(0, 1)
        for ci in range(1, 5):
            dma_w(ci)
        for ns in range(1, NS):
            dma_x(ns, 0)
            dma_x(ns, 1)

        # ---- per-span q/k/v tiles (zero-padded K=128 layout) ----
        qT01 = [qk_pool.tile([P, SPAN], dt_proj, tag="q01", name=f"q01_{i}") for i in range(NS)]
        qT2z = [qk_pool.tile([P, SPAN], dt_proj, tag="q2z", name=f"q2z_{i}") for i in range(NS)]
        vT01 = [qk_pool.tile([P, SPAN], dt_proj, tag="v01", name=f"v01_{i}") for i in range(NS)]
        vT2z = [qk_pool.tile([P, SPAN], dt_proj, tag="v2z", name=f"v2z_{i}") for i in range(NS)]
        kTz = [[kz_pool.tile([P, SPAN], dt_proj, tag="kz", name=f"kz_{h}_{i}")
                for i in range(NS)] for h in range(HL)]

        def zfill(ap):
            nc.gpsimd.memset(ap, 0.0)

        for ns in range(NS):
            zfill(qT2z[ns][HD:P, :])
            zfill(vT2z[ns][0:HD, :])
            zfill(kTz[0][ns][HD:P, :])
            zfill(kTz[1][ns][0:HD, :])
            zfill(kTz[2][ns][HD:P, :])

        # v natural layout: heads 0,1 interleaved per j-tile [v0|1|v1|1], head 2
        # separate [v2|1]; the ones column accumulates the softmax denominator.
        v_nat01 = vnat_pool.tile([P, NT * 2 * (HD + 1)], dt_p, tag="vnat01")
        v_nat2 = vnat_pool.tile([P, NT * (HD + 1)], dt_p, tag="vnat2")
        c01 = v_nat01[:].rearrange("p (t c) -> p t c", c=HD + 1)[:, :, HD]
        c2 = v_nat2[:].rearrange("p (t c) -> p t c", c=HD + 1)[:, :, HD]
        nc.gpsimd.memset(c01, 1.0)
        nc.gpsimd.memset(c2, 1.0)

        def vnat(h, jt):
            if h < 2:
                b = jt * 2 * (HD + 1) + h * (HD + 1)
                return v_nat01[:, b:b + HD + 1]
            b = jt * (HD + 1)
            return v_nat2[:, b:b + HD + 1]

        # ---- warmup: keep the PE busy while the first DMAs land ----
        warm = ps_pj.tile([P, SPAN], F32, tag="ps_pj", name="warm")

        def warmup(n):
            for _ in range(n):
                nc.tensor.matmul(warm[:], zeros_r[:, 0:P], zeros_r[:],
                                 start=True, stop=True)

        warmup(WARMUP_N)

        # ---- projections: qk chunks (needed before attn(s) starts) and v
        # chunks + transposes (needed only by attn(s)'s DIAGONAL j-tiles,
        # so they can fill deep into attn(s) itself) ----
        m_chunks = ((0, P, "q01"), (1, P, "k01"), (2, P, "k2q2"),
                    (3, P, "v01"), (4, HD, "v2"))

        def chunk_op(ns, ci, msz, what, midfill=0):
            pt = ps_pj.tile([msz, SPAN], F32, tag="ps_pj", name=f"pj_{ns}_{what}")
            for kt in range(KT):
                if midfill and kt == KH:
                    warmup(midfill)
                nc.tensor.matmul(
                    pt[:], w_slice(ci, kt), x_slice(ns, kt),
                    start=(kt == 0), stop=(kt == KT - 1))
            if what == "q01":
                nc.vector.tensor_copy(qT01[ns][:], pt[:])
            elif what == "k01":
                nc.vector.tensor_copy(kTz[0][ns][0:HD, :], pt[0:HD, :])
                nc.vector.tensor_copy(kTz[1][ns][HD:P, :], pt[HD:P, :])
            elif what == "k2q2":
                nc.vector.tensor_copy(kTz[2][ns][0:HD, :], pt[0:HD, :])
                nc.vector.tensor_copy(qT2z[ns][0:HD, :], pt[HD:P, :])
            elif what == "v01":
                nc.vector.tensor_copy(vT01[ns][:], pt[:])
            else:
                nc.vector.tensor_copy(vT2z[ns][HD:P, :], pt[:])

        def transp01_op(ns, c):
            jt = ns * CPS + c
            tp = ps_pj.tile([P, P], dt_proj, tag="ps_pj", name=f"tp_{jt}")
            nc.tensor.transpose(tp[:], vT01[ns][:, c * P:(c + 1) * P],
                                ident_r[:])
            nc.vector.tensor_copy(
                v_nat01[:].rearrange("p (t c) -> p t c", c=HD + 1)[
                    :, 2 * jt:2 * jt + 2, 0:HD],
                tp[:].rearrange("p (t c) -> p t c", c=HD))

        def transp2_op(ns, c):
            jt = ns * CPS + c
            tp2 = ps_pj.tile([P, P], dt_proj, tag="ps_pj", name=f"tp2_{jt}")
            nc.tensor.transpose(tp2[:], vT2z[ns][:, c * P:(c + 1) * P],
                                ident_r[:])
            nc.vector.tensor_copy(
                v_nat2[:, jt * (HD + 1):jt * (HD + 1) + HD], tp2[:, HD:P])

        def qk_groups(ns):
            return [[lambda a=ci, b=msz, w=what: chunk_op(ns, a, b, w)]
                    for (ci, msz, what) in m_chunks[:3]]

        def v_groups(ns):
            # transposes ride in bursts of 4: a lone transpose between
            # attention matmuls costs two PE mode switches (measured +34ns
            # on every neighboring matmul when fully scattered)
            gs = [[lambda a=ci, b=msz, w=what: chunk_op(ns, a, b, w)]
                  for (ci, msz, what) in m_chunks[3:]]
            for c0 in (0, 2):
                gs.append([lambda c=c, t=t: t(ns, c) for c in (c0, c0 + 1)
                           for t in (transp01_op, transp2_op)])
            return gs

        # due-point scheduler: each deferred proj GROUP gets an absolute pav
        # insertion-point index; drained at-or-before that point. Points:
        # attn(0): 0-7, attn(1): 8-21, attn(2): 22-41, attn(3): 42-67.
        sched = []
        point = [0]

        def add_filler(groups, p0, p1):
            n = len(groups)
            for j, g in enumerate(groups):
                sched.append((p0 + (p1 - p0) * j // n, g))

        def drain_even():
            while sched and sched[0][0] <= point[0]:
                for op in sched.pop(0)[1]:
                    op()
            point[0] += 1

        def finalize(s, h, av):
            ob = osb_pool.tile([HD + 1, SPAN], F32, tag="osb", name=f"ob{s}_{h}")
            nc.vector.tensor_copy(ob[:], av[:])
            nc.sync.dma_start(
                o[h * (HD + 1):(h + 1) * (HD + 1),
                  s * SPAN:(s + 1) * SPAN], ob[:])

        # ---- attention: heads 0,1 fused pair loop; head 2 solo ----
        def jt_order(s, njt):
            # sequential: full tiles stream b2b, diag chains at the end
            # (interleaving diags mid-stream measured WORSE: +6.7us of
            # pipeline gaps in spans 2-3)
            return list(range(njt))

        def attn01(s):
            njt = CPS * (s + 1)
            order = jt_order(s, njt)
            av0 = ps_av.tile([HD + 1, SPAN], F32, tag="ps_av", name=f"av0_{s}")
            av1 = ps_av.tile([HD + 1, SPAN], F32, tag="ps_av", name=f"av1_{s}")
            live = {}

            def emit_sc(jt):
                c_d = jt - CPS * s
                n0 = max(c_d, 0) * P
                ns_k, ck = jt // CPS, jt % CPS
                sc = ps.tile([P, 2 * SPAN], F32, tag="ps", name=f"sc01_{s}_{jt}")
                nc.tensor.matmul(sc[:, n0:SPAN],
                                 kTz[0][ns_k][:, ck * P:(ck + 1) * P],
                                 qT01[s][:, n0:SPAN], start=True, stop=True)
                nc.tensor.matmul(sc[:, SPAN + n0:2 * SPAN],
                                 kTz[1][ns_k][:, ck * P:(ck + 1) * P],
                                 qT01[s][:, n0:SPAN], start=True, stop=True)
                live[jt] = (sc, n0, c_d >= 0)

            def emit_pav(i):
                jt = order[i]
                sc, n0, diag = live.pop(jt)
                p = ppool.tile([P, 2 * SPAN], dt_p, tag="p", name=f"p01_{s}_{jt}")
                sc3 = sc[:].rearrange("q (t c) -> q t c", c=SPAN)
                p3 = p[:].rearrange("q (t c) -> q t c", c=SPAN)
                nc.scalar.activation(p3[:, :, n0:SPAN], sc3[:, :, n0:SPAN], EXP)
                if diag:
                    nc.vector.tensor_mul(p[:, n0:n0 + P], p[:, n0:n0 + P],
                                         tri16[:])
                    nc.vector.tensor_mul(
                        p[:, SPAN + n0:SPAN + n0 + P],
                        p[:, SPAN + n0:SPAN + n0 + P], tri16[:])
                # safe insertion point: every live sc tile's reader is emitted;
                # proj bursts and the two-ahead sc keep Tensor fed through the
                # exp latency
                drain_even()
                if i + 2 < njt and order[i + 2] not in live:
                    emit_sc(order[i + 2])
                st, sp = (i == 0), (i == njt - 1)
                nc.tensor.matmul(av0[:, n0:SPAN], vnat(0, jt), p[:, n0:SPAN],
                                 start=st, stop=sp)
                nc.tensor.matmul(av1[:, n0:SPAN], vnat(1, jt),
                                 p[:, SPAN + n0:2 * SPAN], start=st, stop=sp)

            emit_sc(order[0])
            if njt > 1:
                emit_sc(order[1])
            for i in range(njt):
                emit_pav(i)
            finalize(s, 0, av0)
            finalize(s, 1, av1)

        def attn2(s):
            # head-2 stream: full tiles processed in PAIRS sharing one
            # [128, 2*SPAN] psum tile and ONE exp instruction (halves the
            # ACT fixed cost); diagonal tiles stay single, woven between
            njt = CPS * (s + 1)
            full = list(range(CPS * s))
            diag = list(range(CPS * s, njt))
            units = [(full[i], full[i + 1]) for i in range(0, len(full), 2)]
            # diag j-tiles pair up too: one exp over the union column range
            # [n0_first:512] per half; the second tile's [n0_first:n0_own)
            # slice exps stale psum (finite old scores) that no av reads
            units.extend(((diag[0], diag[1]), (diag[2], diag[3])))
            av2 = ps_av.tile([HD + 1, SPAN], F32, tag="ps_av", name=f"av2_{s}")
            live = {}

            def emit_sc(u):
                sc = ps.tile([P, 2 * SPAN], F32, tag="ps",
                             name=f"sc2_{s}_{u[0]}")
                n0f = SPAN
                for t, jt in enumerate(u):
                    c_d = jt - CPS * s
                    n0 = max(c_d, 0) * P
                    n0f = min(n0f, n0)
                    ns_k, ck = jt // CPS, jt % CPS
                    nc.tensor.matmul(sc[:, t * SPAN + n0:(t + 1) * SPAN],
                                     kTz[2][ns_k][:, ck * P:(ck + 1) * P],
                                     qT2z[s][:, n0:SPAN], start=True, stop=True)
                live[u] = (sc, n0f)

            def emit_pav(i):
                u = units[i]
                sc, n0, = live.pop(u)
                p = ppool.tile([P, 2 * SPAN], dt_p, tag="p",
                               name=f"p2_{s}_{u[0]}")
                sc3 = sc[:].rearrange("q (t c) -> q t c", c=SPAN)
                p3 = p[:].rearrange("q (t c) -> q t c", c=SPAN)
                nc.scalar.activation(p3[:, :, n0:SPAN], sc3[:, :, n0:SPAN],
                                     EXP)
                for t, jt in enumerate(u):
                    c_d = jt - CPS * s
                    if c_d >= 0:
                        tn0 = t * SPAN + c_d * P
                        nc.vector.tensor_mul(p[:, tn0:tn0 + P],
                                             p[:, tn0:tn0 + P], tri16[:])
                drain_even()
                if i + 2 < len(units) and units[i + 2] not in live:
                    emit_sc(units[i + 2])
                for t, jt in enumerate(u):
                    c_d = jt - CPS * s
                    tn0 = max(c_d, 0) * P
                    st = (i == 0 and t == 0)
                    sp = (i == len(units) - 1 and t == len(u) - 1)
                    nc.tensor.matmul(av2[:, tn0:SPAN], vnat(2, jt),
                                     p[:, t * SPAN + tn0:(t + 1) * SPAN],
                                     start=st, stop=sp)

            emit_sc(units[0])
            if len(units) > 1:
                emit_sc(units[1])
            for i in range(len(units)):
                emit_pav(i)
            finalize(s, 2, av2)

        # span 0 projects standalone (warmup mid-fill covers the kt0-2 ->
        # kt3-5 x-DMA boundary of the first chunk); everything later drains
        # into the attention phases as PE filler, as late as its consumers
        # allow -- qk(s) before attn(s) starts, v(s)+transposes(s) before
        # attn(s)'s first DIAGONAL av -- so the ACT-paced late spans stay
        # PE-dense
        first = True
        for (ci, msz, what) in m_chunks:
            chunk_op(0, ci, msz, what, midfill=WARMUP_MID if first else 0)
            first = False
        for c in range(CPS):
            transp01_op(0, c)
            transp2_op(0, c)
        add_filler(qk_groups(1), 0, 3)
        add_filler(v_groups(1), 3, 12)
        add_filler(qk_groups(2), 12, 19)
        add_filler(v_groups(2), 19, 30)
        add_filler(qk_groups(3), 30, 39)
        add_filler(v_groups(3), 39, 53)
        for s in range(NS):
            attn01(s)
            attn2(s)
        while sched:
            for op in sched.pop(0)[1]:
                op()


_NC_CACHE = {}


def _get_module(dt_proj=DT_PROJ, dt_p=DT_P):
    key = (dt_proj, dt_p)
    if key not in _NC_CACHE:
        nc = bacc.Bacc("TRN2", target_bir_lowering=False, debug=False)
        with tile.TileContext(nc) as tc:
            _build(nc, tc, dt_proj, dt_p)
        nc.compile()
        _NC_CACHE[key] = nc
    return _NC_CACHE[key]


def _in_maps(x, Wq, Wk, Wv):
    maps = []
    xT = [np.ascontiguousarray(
        x[b].T.reshape(KT, P, NS, SPAN).transpose(1, 2, 0, 3).reshape(P, -1)
        .astype(np.float16))
        for b in range(B)]
    WqT, WkT, WvT = Wq.T, Wk.T, Wv.T
    for c in range(N_CORES):
        bc, g = divmod(c, N_CORES // B)
        s0 = g * DL
        wcomb = np.concatenate([
            WqT[:, s0:s0 + P], WkT[:, s0:s0 + P],
            WkT[:, s0 + P:s0 + DL], WqT[:, s0 + P:s0 + DL],
            WvT[:, s0:s0 + P], WvT[:, s0 + P:s0 + DL]], axis=1)
        # pack as (chunk, kt, m): per m-chunk, kt-major
        w3 = wcomb.reshape(KT, P, 3 * DL).transpose(1, 0, 2)  # [P, kt, m]
        parts = []
        for c0, c1 in ((0, P), (P, 2 * P), (2 * P, 3 * P), (3 * P, 4 * P),
                       (4 * P, 4 * P + HD)):
            parts.append(w3[:, :, c0:c1].reshape(P, -1))
        wpk = np.ascontiguousarray(
            np.concatenate(parts, axis=1).astype(np.float16))
        maps.append({
            "xt": xT[bc],
            "wc": wpk,
        })
    return maps


def kernel(x, Wq, Wk, Wv, _trace=False, _tmpdir=None, **_kw):
    x = np.asarray(x, dtype=np.float32)
    Wq = np.asarray(Wq, dtype=np.float32)
    Wk = np.asarray(Wk, dtype=np.float32)
    Wv = np.asarray(Wv, dtype=np.float32)
    assert x.shape == (B, N, D) and Wq.shape == (D, D)

    nc = _get_module()
    res = bass_utils.run_bass_kernel_spmd(
        nc, _in_maps(x, Wq, Wk, Wv), core_ids=list(range(N_CORES)),
        trace=_trace, tmpdir=_tmpdir)
    out = np.empty((B, N, D), np.float32)
    for c in range(N_CORES):
        bc, g = divmod(c, N_CORES // B)
        oT = res.results[c]["o"].astype(np.float64)
        for h in range(HL):
            blk = oT[h * (HD + 1):h * (HD + 1) + HD, :]
            den = oT[h * (HD + 1) + HD, :]
            out[bc, :, g * DL + h * HD:g * DL + (h + 1) * HD] = \
                (blk / den).T.astype(np.float32)
    if _trace:
        return out, res
    return out

